# revision 2
# baseline (speedup 1.0000x reference)
"""Additive attention (Bahdanau) Trainium2 Bass/Tile kernel.

Model (per batch row b):
    q_proj = query @ Wq + bq                       [U]
    v_proj = values[b] @ Wv + bv                   [T, U]
    hidden = tanh(q_proj + v_proj)                 [T, U]
    score  = hidden @ Vw (+ Vb; softmax-invariant) [T]
    score += (1 - mask) * -1e9
    w      = softmax(score)                        [T]
    ctx    = w @ values[b]                         [D]
returns (context [B, D], attention_weights [B, T])

Sharding: data-parallel over batch. B=32 rows -> 4 rows on each of the 8
NeuronCores; all weights replicated. Each core runs the same NEFF (SPMD).

Per-core dataflow (BL=4 local rows):
  - values[b] loaded once to SBUF in natural [T-part, D-free] tiles and kept
    resident for the whole row (used again by the context matmul).
  - PE transposes 128x128 blocks -> valuesT [D-part, T-free] (matmul needs the
    contraction dim D on partitions).
  - v_projT[u, t] = sum_dc Wv_chunk.T @ valuesT_chunk  (PSUM, fp32 accumulate)
  - hiddenT = tanh(v_projT + bias_col) fused on ScalarE (bias is per-partition
    since U sits on partitions).
  - score accumulates into one PSUM tile [128, 16] column-major (t_lo, t_chunk)
    via N=1 matmuls hidT.T @ Vw.
  - softmax is computed column-major WITHOUT max subtraction (|score| <= ~40
    since |hidden|<=1 and Vw is small; masked lanes get -1e9 -> exp -> 0 which
    matches the reference exactly): exp on ACT with accum_out giving per-
    partition sums, a ones-vector matmul reduces across partitions, reciprocal
    on DVE, and a second ones-matmul broadcasts it back to 128 partitions.
  - context[d] = sum_t w_t * values[t, d] via N=1 matmuls with the resident
    natural tiles as stationary operand.
"""

import numpy as np

B, T, D, U = 32, 2048, 1024, 1024
NCORES = 8
BL = B // NCORES  # 4 local batch rows per core
NT = 4            # t-tiles per row, 512 wide each
TT = 512
C4 = TT // 128    # 128-chunks per t-tile
TC = T // 128     # 128-chunks per row (16)
DC = D // 128     # 8
UC = U // 128     # 8
NEG = -1.0e9

_CACHE = {}


def _build_nc(mm_dt_name="float32"):
    import concourse.bacc as bacc
    import concourse.tile as tile
    from concourse import mybir, masks

    f32 = mybir.dt.float32
    i32 = mybir.dt.int32
    mm_dt = getattr(mybir.dt, mm_dt_name)
    AF = mybir.ActivationFunctionType
    ALU = mybir.AluOpType

    def mm(ap):
        # view an fp32 AP under the matmul streaming dtype (e.g. float32r)
        return ap if mm_dt == f32 else ap.bitcast(mm_dt)

    nc = bacc.Bacc("TRN2", target_bir_lowering=False, debug=False)

    q_d = nc.dram_tensor("query", [BL, D], f32, kind="ExternalInput").ap()
    v_d = nc.dram_tensor("values", [BL, T, D], f32, kind="ExternalInput").ap()
    m_d = nc.dram_tensor("mask", [BL, T], i32, kind="ExternalInput").ap()
    wq_d = nc.dram_tensor("Wq", [D, U], f32, kind="ExternalInput").ap()
    bq_d = nc.dram_tensor("bq", [U], f32, kind="ExternalInput").ap()
    wv_d = nc.dram_tensor("Wv", [D, U], f32, kind="ExternalInput").ap()
    bv_d = nc.dram_tensor("bv", [U], f32, kind="ExternalInput").ap()
    vw_d = nc.dram_tensor("Vw", [U, 1], f32, kind="ExternalInput").ap()
    ctx_d = nc.dram_tensor("context", [BL, D], f32, kind="ExternalOutput").ap()
    aw_d = nc.dram_tensor(
        "attention_weights", [BL, T], f32, kind="ExternalOutput"
    ).ap()

    with tile.TileContext(nc) as tc:
        with (
            tc.tile_pool(name="const", bufs=1) as constp,
            tc.tile_pool(name="wpool", bufs=1) as wpool,
            tc.tile_pool(name="vals", bufs=6) as valsp,
            tc.tile_pool(name="vT", bufs=9) as vTp,
            tc.tile_pool(name="hid", bufs=9) as hidp,
            tc.tile_pool(name="small", bufs=2) as smallp,
            tc.tile_pool(name="ptr", bufs=2, space="PSUM") as ptrp,
            tc.tile_pool(name="pv", bufs=2, space="PSUM") as pvp,
            tc.tile_pool(name="ps", bufs=2, space="PSUM") as psp,
            tc.tile_pool(name="pc", bufs=1, space="PSUM") as pcp,
            tc.tile_pool(name="pm", bufs=1, space="PSUM") as pmp,
        ):
            # ---------------- constants / preamble ----------------
            identity = constp.tile([128, 128], f32)
            masks.make_identity(nc, identity[:])
            ones_col = constp.tile([128, 1], f32)
            nc.gpsimd.memset(ones_col[:], 1.0)
            ones_row = constp.tile([1, 128], f32)
            nc.gpsimd.memset(ones_row[:], 1.0)

            vw_sb = constp.tile([128, UC], f32)  # [u_lo, uc]
            nc.sync.dma_start(
                vw_sb[:], vw_d.rearrange("(c l) one -> l (c one)", l=128)
            )
            bqv = constp.tile([128, UC], f32)  # bq + bv, [u_lo, uc]
            tmpb = constp.tile([128, UC], f32)
            nc.sync.dma_start(bqv[:], bq_d.rearrange("(c l) -> l c", l=128))
            nc.sync.dma_start(tmpb[:], bv_d.rearrange("(c l) -> l c", l=128))
            nc.vector.tensor_add(bqv[:], bqv[:], tmpb[:])

            qT = constp.tile([128, DC, BL], f32)  # [d_lo, dc, b]
            for dc in range(DC):
                nc.sync.dma_start(
                    qT[:, dc, :],
                    q_d[:, dc * 128:(dc + 1) * 128].rearrange("b l -> l b"),
                )

            # mask addend, column-major per row: (mask - 1) * 1e9
            mcol_i = constp.tile([128, BL, TC], i32)
            for b in range(BL):
                nc.sync.dma_start(
                    mcol_i[:, b, :], m_d[b].rearrange("(c l) -> l c", l=128)
                )
            addend = constp.tile([128, BL, TC], f32)
            nc.vector.tensor_copy(addend[:], mcol_i[:])  # int32 -> fp32
            nc.vector.tensor_scalar(
                out=addend[:], in0=addend[:], scalar1=1.0e9, scalar2=NEG,
                op0=ALU.mult, op1=ALU.add,
            )

            # bias[u_lo, uc, b] = q_proj[b, u] + bq[u] + bv[u], via PE
            bias = constp.tile([128, UC, BL], f32)
            for uh in range(2):  # stream Wq in two halves to save SBUF
                wq_sb = wpool.tile([128, DC, U // 2], f32, tag="wq")
                nc.sync.dma_start(
                    wq_sb[:],
                    wq_d[:, uh * 512:(uh + 1) * 512].rearrange(
                        "(c l) u -> l c u", l=128
                    ),
                )
                for uq in range(UC // 2):
                    uc = uh * (UC // 2) + uq
                    pq = ptrp.tile([128, BL], f32, tag="ptr")
                    for dc in range(DC):
                        nc.tensor.matmul(
                            pq[:],
                            mm(wq_sb[:, dc, uq * 128:(uq + 1) * 128]),
                            mm(qT[:, dc, :]),
                            start=(dc == 0),
                            stop=(dc == DC - 1),
                        )
                    nc.vector.tensor_scalar_add(
                        bias[:, uc, :], pq[:], bqv[:, uc:uc + 1]
                    )

            wv_sb = wpool.tile([128, DC, U], f32, tag="wv")  # [d_lo, dc, u]
            nc.sync.dma_start(
                wv_sb[:], wv_d.rearrange("(c l) u -> l c u", l=128)
            )

            # ---------------- main loop over local batch rows ----------------
            for b in range(BL):
                vals_b = []
                for tt in range(NT):
                    vt = valsp.tile([128, C4, D], f32, tag="vals")
                    nc.sync.dma_start(
                        vt[:],
                        v_d[b, tt * TT:(tt + 1) * TT, :].rearrange(
                            "(c p) d -> p c d", p=128
                        ),
                    )
                    vals_b.append(vt)

                psc = psp.tile([128, TC], f32, tag="ps")  # score, column-major
                first_score = True
                for tt in range(NT):
                    # --- transpose values tile -> [d, t] ---
                    vT_tiles = []
                    for dc in range(DC):
                        ptr_t = ptrp.tile([128, TT], f32, tag="ptr")
                        for j in range(C4):
                            nc.tensor.matmul(
                                ptr_t[:, j * 128:(j + 1) * 128],
                                vals_b[tt][:, j, dc * 128:(dc + 1) * 128],
                                identity[:],
                                is_transpose=True,
                                start=(j == 0),
                                stop=(j == C4 - 1),
                            )
                        vTt = vTp.tile([128, TT], f32, tag="vT")
                        nc.vector.tensor_copy(vTt[:], ptr_t[:])
                        vT_tiles.append(vTt)
                    # --- v_projT + tanh ---
                    hid_tiles = []
                    for uc in range(UC):
                        pvt = pvp.tile([128, TT], f32, tag="pv")
                        for dc in range(DC):
                            nc.tensor.matmul(
                                pvt[:],
                                mm(wv_sb[:, dc, uc * 128:(uc + 1) * 128]),
                                mm(vT_tiles[dc][:]),
                                start=(dc == 0),
                                stop=(dc == DC - 1),
                            )
                        ht = hidp.tile([128, TT], f32, tag="hid")
                        nc.scalar.activation(
                            ht[:], pvt[:], AF.Tanh, bias=bias[:, uc, b:b + 1]
                        )
                        hid_tiles.append(ht)
                    # --- score (N=1 matmuls, one PSUM group per row) ---
                    for j in range(C4):
                        for uc in range(UC):
                            nc.tensor.matmul(
                                psc[:, tt * C4 + j: tt * C4 + j + 1],
                                hid_tiles[uc][:, j * 128:(j + 1) * 128],
                                vw_sb[:, uc:uc + 1],
                                start=first_score,
                                stop=(
                                    tt == NT - 1 and j == C4 - 1 and uc == UC - 1
                                ),
                            )
                            first_score = False

                # --- masked softmax, column-major, no max subtraction ---
                masked = smallp.tile([128, TC], f32, tag="masked")
                nc.vector.tensor_add(masked[:], psc[:], addend[:, b, :])
                expc = smallp.tile([128, TC], f32, tag="expc")
                partial = smallp.tile([128, 1], f32, tag="partial")
                nc.scalar.activation(
                    expc[:], masked[:], AF.Exp, accum_out=partial[:]
                )
                ptot = pmp.tile([1, 1], f32, tag="pm")
                nc.tensor.matmul(
                    ptot[:], partial[:], ones_col[:], start=True, stop=True
                )
                recip = smallp.tile([1, 1], f32, tag="recip")
                nc.vector.reciprocal(recip[:], ptot[:])
                pbc = pmp.tile([128, 1], f32, tag="pm")
                nc.tensor.matmul(
                    pbc[:], ones_row[:], recip[:], start=True, stop=True
                )
                rbc = smallp.tile([128, 1], f32, tag="rbc")
                nc.vector.tensor_copy(rbc[:], pbc[:])
                wT = smallp.tile([128, TC], f32, tag="wT")
                nc.vector.tensor_scalar_mul(wT[:], expc[:], rbc[:])
                nc.sync.dma_start(
                    aw_d[b].rearrange("(c l) -> l c", l=128), wT[:]
                )

                # --- context ---
                pctx = pcp.tile([128, DC], f32, tag="pc")
                first = True
                for dc in range(DC):
                    for c in range(TC):
                        nc.tensor.matmul(
                            pctx[:, dc:dc + 1],
                            vals_b[c // C4][:, c % C4, dc * 128:(dc + 1) * 128],
                            wT[:, c:c + 1],
                            start=first,
                            stop=(dc == DC - 1 and c == TC - 1),
                        )
                        first = False
                ctx_sb = smallp.tile([128, DC], f32, tag="ctxsb")
                nc.vector.tensor_copy(ctx_sb[:], pctx[:])
                nc.sync.dma_start(
                    ctx_d[b].rearrange("(c l) -> l c", l=128), ctx_sb[:]
                )

    nc.compile()
    return nc


def get_nc():
    if "nc" not in _CACHE:
        _CACHE["nc"] = _build_nc()
    return _CACHE["nc"]


def make_in_maps(query, values, mask, Wq, bq, Wv, bv, Vw):
    in_maps = []
    for c in range(NCORES):
        sl = slice(c * BL, (c + 1) * BL)
        in_maps.append({
            "query": np.ascontiguousarray(query[sl]),
            "values": np.ascontiguousarray(values[sl]),
            "mask": np.ascontiguousarray(mask[sl]),
            "Wq": Wq, "bq": bq, "Wv": Wv, "bv": bv, "Vw": Vw,
        })
    return in_maps


def kernel(query, values, mask, Wq, bq, Wv, bv, Vw, Vb):
    from concourse.bass_utils import run_bass_kernel_spmd

    query = np.asarray(query, np.float32)
    values = np.asarray(values, np.float32)
    mask = np.asarray(mask, np.int32)
    Wq = np.asarray(Wq, np.float32)
    bq = np.asarray(bq, np.float32)
    Wv = np.asarray(Wv, np.float32)
    bv = np.asarray(bv, np.float32)
    Vw = np.asarray(Vw, np.float32)
    # Vb shifts every score equally -> softmax-invariant; not needed on device.

    nc = get_nc()
    in_maps = make_in_maps(query, values, mask, Wq, bq, Wv, bv, Vw)
    res = run_bass_kernel_spmd(nc, in_maps, list(range(NCORES))).results
    context = np.concatenate([r["context"] for r in res], axis=0)
    aw = np.concatenate([r["attention_weights"] for r in res], axis=0)
    return context, aw


# revision 5
# speedup vs baseline: 10.0360x; 10.0360x over previous
"""Additive attention (Bahdanau) Trainium2 Bass/Tile kernel.

Model (per batch row b):
    q_proj = query @ Wq + bq                       [U]
    v_proj = values[b] @ Wv + bv                   [T, U]
    hidden = tanh(q_proj + v_proj)                 [T, U]
    score  = hidden @ Vw (+ Vb; softmax-invariant) [T]
    score += (1 - mask) * -1e9
    w      = softmax(score)                        [T]
    ctx    = w @ values[b]                         [D]
returns (context [B, D], attention_weights [B, T])

Sharding: data-parallel over batch. B=32 rows -> 4 rows on each of the 8
NeuronCores; all weights replicated. Each core runs the same NEFF (SPMD).

Per-core dataflow (BL=4 local rows):
  - values[b] loaded once to SBUF in natural [T-part, D-free] tiles and kept
    resident for the whole row (used again by the context matmul).
  - PE transposes 128x128 blocks -> valuesT [D-part, T-free] (matmul needs the
    contraction dim D on partitions).
  - v_projT[u, t] = sum_dc Wv_chunk.T @ valuesT_chunk  (PSUM, fp32 accumulate)
  - hiddenT = tanh(v_projT + bias_col) fused on ScalarE (bias is per-partition
    since U sits on partitions).
  - score accumulates into one PSUM tile [128, 16] column-major (t_lo, t_chunk)
    via N=1 matmuls hidT.T @ Vw.
  - softmax is computed column-major WITHOUT max subtraction (|score| <= ~40
    since |hidden|<=1 and Vw is small; masked lanes get -1e9 -> exp -> 0 which
    matches the reference exactly): exp on ACT with accum_out giving per-
    partition sums, a ones-vector matmul reduces across partitions, reciprocal
    on DVE, and a second ones-matmul broadcasts it back to 128 partitions.
  - context[d] = sum_t w_t * values[t, d] via N=1 matmuls with the resident
    natural tiles as stationary operand.
"""

import numpy as np

B, T, D, U = 32, 2048, 1024, 1024
NCORES = 8
BL = B // NCORES  # 4 local batch rows per core
NT = 4            # t-tiles per row, 512 wide each
TT = 512
C4 = TT // 128    # 128-chunks per t-tile
TC = T // 128     # 128-chunks per row (16)
DC = D // 128     # 8
UC = U // 128     # 8
NEG = -1.0e9

_CACHE = {}


def _build_nc(mm_dt_name="float32", repeat=1):
    import concourse.bacc as bacc
    import concourse.tile as tile
    from concourse import mybir, masks

    f32 = mybir.dt.float32
    i32 = mybir.dt.int32
    mm_dt = getattr(mybir.dt, mm_dt_name)
    AF = mybir.ActivationFunctionType
    ALU = mybir.AluOpType

    def mm(ap):
        # view an fp32 AP under the matmul streaming dtype (e.g. float32r)
        return ap if mm_dt == f32 else ap.bitcast(mm_dt)

    nc = bacc.Bacc("TRN2", target_bir_lowering=False, debug=False)

    q_d = nc.dram_tensor("query", [BL, D], f32, kind="ExternalInput").ap()
    v_d = nc.dram_tensor("values", [BL, T, D], f32, kind="ExternalInput").ap()
    m_d = nc.dram_tensor("mask", [BL, T], i32, kind="ExternalInput").ap()
    wq_d = nc.dram_tensor("Wq", [D, U], f32, kind="ExternalInput").ap()
    bq_d = nc.dram_tensor("bq", [U], f32, kind="ExternalInput").ap()
    wv_d = nc.dram_tensor("Wv", [D, U], f32, kind="ExternalInput").ap()
    bv_d = nc.dram_tensor("bv", [U], f32, kind="ExternalInput").ap()
    vw_d = nc.dram_tensor("Vw", [U, 1], f32, kind="ExternalInput").ap()
    ctx_d = nc.dram_tensor("context", [BL, D], f32, kind="ExternalOutput").ap()
    aw_d = nc.dram_tensor(
        "attention_weights", [BL, T], f32, kind="ExternalOutput"
    ).ap()

    with tile.TileContext(nc) as tc:
        with (
            tc.tile_pool(name="const", bufs=1) as constp,
            tc.tile_pool(name="wpool", bufs=1) as wpool,
            tc.tile_pool(name="vals", bufs=6) as valsp,
            tc.tile_pool(name="vT", bufs=9) as vTp,
            tc.tile_pool(name="hid", bufs=9) as hidp,
            tc.tile_pool(name="small", bufs=2) as smallp,
            tc.tile_pool(name="ptr", bufs=2, space="PSUM") as ptrp,
            tc.tile_pool(name="pv", bufs=2, space="PSUM") as pvp,
            tc.tile_pool(name="ps", bufs=2, space="PSUM") as psp,
            tc.tile_pool(name="pc", bufs=1, space="PSUM") as pcp,
            tc.tile_pool(name="pm", bufs=1, space="PSUM") as pmp,
        ):
            # ---------------- constants / preamble ----------------
            identity = constp.tile([128, 128], f32)
            masks.make_identity(nc, identity[:])
            ones_col = constp.tile([128, 1], f32)
            nc.gpsimd.memset(ones_col[:], 1.0)
            ones_row = constp.tile([1, 128], f32)
            nc.gpsimd.memset(ones_row[:], 1.0)

            vw_sb = constp.tile([128, UC], f32)  # [u_lo, uc]
            nc.sync.dma_start(
                vw_sb[:], vw_d.rearrange("(c l) one -> l (c one)", l=128)
            )
            bqv = constp.tile([128, UC], f32)  # bq + bv, [u_lo, uc]
            tmpb = constp.tile([128, UC], f32)
            nc.sync.dma_start(bqv[:], bq_d.rearrange("(c l) -> l c", l=128))
            nc.sync.dma_start(tmpb[:], bv_d.rearrange("(c l) -> l c", l=128))
            nc.vector.tensor_add(bqv[:], bqv[:], tmpb[:])

            qT = constp.tile([128, DC, BL], f32)  # [d_lo, dc, b]
            for dc in range(DC):
                nc.sync.dma_start(
                    qT[:, dc, :],
                    q_d[:, dc * 128:(dc + 1) * 128].rearrange("b l -> l b"),
                )

            # mask addend, column-major per row: (mask - 1) * 1e9
            mcol_i = constp.tile([128, BL, TC], i32)
            for b in range(BL):
                nc.sync.dma_start(
                    mcol_i[:, b, :], m_d[b].rearrange("(c l) -> l c", l=128)
                )
            addend = constp.tile([128, BL, TC], f32)
            nc.vector.tensor_copy(addend[:], mcol_i[:])  # int32 -> fp32
            nc.vector.tensor_scalar(
                out=addend[:], in0=addend[:], scalar1=1.0e9, scalar2=NEG,
                op0=ALU.mult, op1=ALU.add,
            )

            # bias[u_lo, uc, b] = q_proj[b, u] + bq[u] + bv[u], via PE
            bias = constp.tile([128, UC, BL], f32)
            for uh in range(2):  # stream Wq in two halves to save SBUF
                wq_sb = wpool.tile([128, DC, U // 2], f32, tag="wq")
                nc.sync.dma_start(
                    wq_sb[:],
                    wq_d[:, uh * 512:(uh + 1) * 512].rearrange(
                        "(c l) u -> l c u", l=128
                    ),
                )
                for uq in range(UC // 2):
                    uc = uh * (UC // 2) + uq
                    pq = ptrp.tile([128, BL], f32, tag="ptr")
                    for dc in range(DC):
                        nc.tensor.matmul(
                            pq[:],
                            mm(wq_sb[:, dc, uq * 128:(uq + 1) * 128]),
                            mm(qT[:, dc, :]),
                            start=(dc == 0),
                            stop=(dc == DC - 1),
                        )
                    nc.vector.tensor_scalar_add(
                        bias[:, uc, :], pq[:], bqv[:, uc:uc + 1]
                    )

            wv_sb = wpool.tile([128, DC, U], f32, tag="wv")  # [d_lo, dc, u]
            nc.sync.dma_start(
                wv_sb[:], wv_d.rearrange("(c l) u -> l c u", l=128)
            )

            # ---------------- main loop over local batch rows ----------------
            for b in [bb % BL for bb in range(BL * repeat)]:
                vals_b = []
                for tt in range(NT):
                    vt = valsp.tile([128, C4, D], f32, tag="vals")
                    nc.sync.dma_start(
                        vt[:],
                        v_d[b, tt * TT:(tt + 1) * TT, :].rearrange(
                            "(c p) d -> p c d", p=128
                        ),
                    )
                    vals_b.append(vt)

                psc = psp.tile([128, TC], f32, tag="ps")  # score, column-major
                first_score = True
                for tt in range(NT):
                    # --- transpose values tile -> [d, t] ---
                    vT_tiles = []
                    for dc in range(DC):
                        ptr_t = ptrp.tile([128, TT], f32, tag="ptr")
                        for j in range(C4):
                            nc.tensor.matmul(
                                ptr_t[:, j * 128:(j + 1) * 128],
                                vals_b[tt][:, j, dc * 128:(dc + 1) * 128],
                                identity[:],
                                is_transpose=True,
                                start=(j == 0),
                                stop=(j == C4 - 1),
                            )
                        vTt = vTp.tile([128, TT], f32, tag="vT")
                        nc.vector.tensor_copy(vTt[:], ptr_t[:])
                        vT_tiles.append(vTt)
                    # --- v_projT + tanh ---
                    hid_tiles = []
                    for uc in range(UC):
                        pvt = pvp.tile([128, TT], f32, tag="pv")
                        for dc in range(DC):
                            nc.tensor.matmul(
                                pvt[:],
                                mm(wv_sb[:, dc, uc * 128:(uc + 1) * 128]),
                                mm(vT_tiles[dc][:]),
                                start=(dc == 0),
                                stop=(dc == DC - 1),
                            )
                        ht = hidp.tile([128, TT], f32, tag="hid")
                        nc.scalar.activation(
                            ht[:], pvt[:], AF.Tanh, bias=bias[:, uc, b:b + 1]
                        )
                        hid_tiles.append(ht)
                    # --- score (N=1 matmuls, one PSUM group per row) ---
                    for j in range(C4):
                        for uc in range(UC):
                            nc.tensor.matmul(
                                psc[:, tt * C4 + j: tt * C4 + j + 1],
                                hid_tiles[uc][:, j * 128:(j + 1) * 128],
                                vw_sb[:, uc:uc + 1],
                                start=first_score,
                                stop=(
                                    tt == NT - 1 and j == C4 - 1 and uc == UC - 1
                                ),
                            )
                            first_score = False

                # --- masked softmax, column-major, no max subtraction ---
                masked = smallp.tile([128, TC], f32, tag="masked")
                nc.vector.tensor_add(masked[:], psc[:], addend[:, b, :])
                expc = smallp.tile([128, TC], f32, tag="expc")
                partial = smallp.tile([128, 1], f32, tag="partial")
                nc.scalar.activation(
                    expc[:], masked[:], AF.Exp, accum_out=partial[:]
                )
                ptot = pmp.tile([1, 1], f32, tag="pm")
                nc.tensor.matmul(
                    ptot[:], partial[:], ones_col[:], start=True, stop=True
                )
                recip = smallp.tile([1, 1], f32, tag="recip")
                nc.vector.reciprocal(recip[:], ptot[:])
                pbc = pmp.tile([128, 1], f32, tag="pm")
                nc.tensor.matmul(
                    pbc[:], ones_row[:], recip[:], start=True, stop=True
                )
                rbc = smallp.tile([128, 1], f32, tag="rbc")
                nc.vector.tensor_copy(rbc[:], pbc[:])
                wT = smallp.tile([128, TC], f32, tag="wT")
                nc.vector.tensor_scalar_mul(wT[:], expc[:], rbc[:])
                nc.sync.dma_start(
                    aw_d[b].rearrange("(c l) -> l c", l=128), wT[:]
                )

                # --- context ---
                pctx = pcp.tile([128, DC], f32, tag="pc")
                first = True
                for dc in range(DC):
                    for c in range(TC):
                        nc.tensor.matmul(
                            pctx[:, dc:dc + 1],
                            vals_b[c // C4][:, c % C4, dc * 128:(dc + 1) * 128],
                            wT[:, c:c + 1],
                            start=first,
                            stop=(dc == DC - 1 and c == TC - 1),
                        )
                        first = False
                ctx_sb = smallp.tile([128, DC], f32, tag="ctxsb")
                nc.vector.tensor_copy(ctx_sb[:], pctx[:])
                nc.sync.dma_start(
                    ctx_d[b].rearrange("(c l) -> l c", l=128), ctx_sb[:]
                )

    nc.compile()
    return nc


MM_DT_DEFAULT = "float32"


def get_nc():
    if "nc" not in _CACHE:
        import os
        _CACHE["nc"] = _build_nc(os.environ.get("MM_DT", MM_DT_DEFAULT))
    return _CACHE["nc"]


def make_in_maps(query, values, mask, Wq, bq, Wv, bv, Vw):
    in_maps = []
    for c in range(NCORES):
        sl = slice(c * BL, (c + 1) * BL)
        in_maps.append({
            "query": np.ascontiguousarray(query[sl]),
            "values": np.ascontiguousarray(values[sl]),
            "mask": np.ascontiguousarray(mask[sl]),
            "Wq": Wq, "bq": bq, "Wv": Wv, "bv": bv, "Vw": Vw,
        })
    return in_maps


def kernel(query, values, mask, Wq, bq, Wv, bv, Vw, Vb):
    from concourse.bass_utils import run_bass_kernel_spmd

    query = np.asarray(query, np.float32)
    values = np.asarray(values, np.float32)
    mask = np.asarray(mask, np.int32)
    Wq = np.asarray(Wq, np.float32)
    bq = np.asarray(bq, np.float32)
    Wv = np.asarray(Wv, np.float32)
    bv = np.asarray(bv, np.float32)
    Vw = np.asarray(Vw, np.float32)
    # Vb shifts every score equally -> softmax-invariant; not needed on device.

    nc = get_nc()
    in_maps = make_in_maps(query, values, mask, Wq, bq, Wv, bv, Vw)
    res = run_bass_kernel_spmd(nc, in_maps, list(range(NCORES))).results
    context = np.concatenate([r["context"] for r in res], axis=0)
    aw = np.concatenate([r["attention_weights"] for r in res], axis=0)
    return context, aw


# revision 10
# speedup vs baseline: 11.3554x; 1.1315x over previous
"""Additive attention (Bahdanau) Trainium2 Bass/Tile kernel.

Model (per batch row b):
    q_proj = query @ Wq + bq                       [U]
    v_proj = values[b] @ Wv + bv                   [T, U]
    hidden = tanh(q_proj + v_proj)                 [T, U]
    score  = hidden @ Vw (+ Vb; softmax-invariant) [T]
    score += (1 - mask) * -1e9
    w      = softmax(score)                        [T]
    ctx    = w @ values[b]                         [D]
returns (context [B, D], attention_weights [B, T])

Sharding: data-parallel over batch. B=32 rows -> 4 rows on each of the 8
NeuronCores; all weights replicated. Each core runs the same NEFF (SPMD).

Per-core dataflow (BL=4 local rows):
  - values[b] loaded once to SBUF in natural [T-part, D-free] tiles and kept
    resident for the whole row (used again by the context matmul).
  - PE transposes 128x128 blocks -> valuesT [D-part, T-free] (matmul needs the
    contraction dim D on partitions).
  - v_projT[u, t] = sum_dc Wv_chunk.T @ valuesT_chunk  (PSUM, fp32 accumulate)
  - hiddenT = tanh(v_projT + bias_col) fused on ScalarE (bias is per-partition
    since U sits on partitions).
  - score accumulates into one PSUM tile [128, 16] column-major (t_lo, t_chunk)
    via N=1 matmuls hidT.T @ Vw.
  - softmax is computed column-major WITHOUT max subtraction (|score| <= ~40
    since |hidden|<=1 and Vw is small; masked lanes get -1e9 -> exp -> 0 which
    matches the reference exactly): exp on ACT with accum_out giving per-
    partition sums, a ones-vector matmul reduces across partitions, reciprocal
    on DVE, and a second ones-matmul broadcasts it back to 128 partitions.
  - context[d] = sum_t w_t * values[t, d] via N=1 matmuls with the resident
    natural tiles as stationary operand.
"""

import numpy as np

B, T, D, U = 32, 2048, 1024, 1024
NCORES = 8
BL = B // NCORES  # 4 local batch rows per core
NT = 4            # t-tiles per row, 512 wide each
TT = 512
C4 = TT // 128    # 128-chunks per t-tile
TC = T // 128     # 128-chunks per row (16)
DC = D // 128     # 8
UC = U // 128     # 8
NEG = -1.0e9

_CACHE = {}


def _build_nc(mm_dt_name="float32", repeat=1):
    import concourse.bacc as bacc
    import concourse.tile as tile
    from concourse import mybir, masks

    f32 = mybir.dt.float32
    i32 = mybir.dt.int32
    mm_dt = getattr(mybir.dt, mm_dt_name)
    AF = mybir.ActivationFunctionType
    ALU = mybir.AluOpType

    nc = bacc.Bacc("TRN2", target_bir_lowering=False, debug=False)

    q_d = nc.dram_tensor("query", [BL, D], f32, kind="ExternalInput").ap()
    v_d = nc.dram_tensor("values", [BL, T, D], f32, kind="ExternalInput").ap()
    m_d = nc.dram_tensor("mask", [BL, T], i32, kind="ExternalInput").ap()
    wq_d = nc.dram_tensor("Wq", [D, U], f32, kind="ExternalInput").ap()
    bq_d = nc.dram_tensor("bq", [U], f32, kind="ExternalInput").ap()
    wv_d = nc.dram_tensor("Wv", [D, U], f32, kind="ExternalInput").ap()
    bv_d = nc.dram_tensor("bv", [U], f32, kind="ExternalInput").ap()
    vw_d = nc.dram_tensor("Vw", [U, 1], f32, kind="ExternalInput").ap()
    ctx_d = nc.dram_tensor("context", [BL, D], f32, kind="ExternalOutput").ap()
    aw_d = nc.dram_tensor(
        "attention_weights", [BL, T], f32, kind="ExternalOutput"
    ).ap()

    with tile.TileContext(nc) as tc:
        with (
            tc.tile_pool(name="const", bufs=1) as constp,
            tc.tile_pool(name="wpool", bufs=1) as wpool,
            tc.tile_pool(name="vals", bufs=6) as valsp,
            tc.tile_pool(name="vT", bufs=9) as vTp,
            tc.tile_pool(name="hid", bufs=9) as hidp,
            tc.tile_pool(name="small", bufs=2) as smallp,
            tc.tile_pool(name="ptr", bufs=2, space="PSUM") as ptrp,
            tc.tile_pool(name="pv", bufs=2, space="PSUM") as pvp,
            tc.tile_pool(name="ps", bufs=2, space="PSUM") as psp,
            tc.tile_pool(name="pc", bufs=1, space="PSUM") as pcp,
            tc.tile_pool(name="pm", bufs=1, space="PSUM") as pmp,
        ):
            # ---------------- constants / preamble ----------------
            identity = constp.tile([128, 128], f32)
            masks.make_identity(nc, identity[:])
            ones_col = constp.tile([128, 1], f32)
            nc.gpsimd.memset(ones_col[:], 1.0)
            ones_row = constp.tile([1, 128], f32)
            nc.gpsimd.memset(ones_row[:], 1.0)

            vw_sb = constp.tile([128, UC], f32)  # [u_lo, uc]
            nc.sync.dma_start(
                vw_sb[:], vw_d.rearrange("(c l) one -> l (c one)", l=128)
            )
            bqv = constp.tile([128, UC], f32)  # bq + bv, [u_lo, uc]
            tmpb = constp.tile([128, UC], f32)
            nc.sync.dma_start(bqv[:], bq_d.rearrange("(c l) -> l c", l=128))
            nc.sync.dma_start(tmpb[:], bv_d.rearrange("(c l) -> l c", l=128))
            nc.vector.tensor_add(bqv[:], bqv[:], tmpb[:])

            qT = constp.tile([128, DC, BL], f32)  # [d_lo, dc, b]
            for dc in range(DC):
                nc.sync.dma_start(
                    qT[:, dc, :],
                    q_d[:, dc * 128:(dc + 1) * 128].rearrange("b l -> l b"),
                )

            # mask addend, column-major per row: (mask - 1) * 1e9
            mcol_i = constp.tile([128, BL, TC], i32)
            for b in range(BL):
                nc.sync.dma_start(
                    mcol_i[:, b, :], m_d[b].rearrange("(c l) -> l c", l=128)
                )
            addend = constp.tile([128, BL, TC], f32)
            nc.vector.tensor_copy(addend[:], mcol_i[:])  # int32 -> fp32
            nc.vector.tensor_scalar(
                out=addend[:], in0=addend[:], scalar1=1.0e9, scalar2=NEG,
                op0=ALU.mult, op1=ALU.add,
            )

            # bias[u_lo, uc, b] = q_proj[b, u] + bq[u] + bv[u], via PE
            bias = constp.tile([128, UC, BL], f32)
            for uh in range(2):  # stream Wq in two halves to save SBUF
                wq_sb = wpool.tile([128, DC, U // 2], f32, tag="wq")
                nc.sync.dma_start(
                    wq_sb[:],
                    wq_d[:, uh * 512:(uh + 1) * 512].rearrange(
                        "(c l) u -> l c u", l=128
                    ),
                )
                for uq in range(UC // 2):
                    uc = uh * (UC // 2) + uq
                    pq = ptrp.tile([128, BL], f32, tag="ptr")
                    for dc in range(DC):
                        nc.tensor.matmul(
                            pq[:],
                            wq_sb[:, dc, uq * 128:(uq + 1) * 128],
                            qT[:, dc, :],
                            start=(dc == 0),
                            stop=(dc == DC - 1),
                        )
                    nc.vector.tensor_scalar_add(
                        bias[:, uc, :], pq[:], bqv[:, uc:uc + 1]
                    )

            # Wv as matmul stationary operand. For float32r the matmul inputs
            # must come from a producer that *rounds* to f32r (BIR verifier
            # rule) -> stage fp32 chunks through a DVE cast.
            wv_sb = wpool.tile([128, DC, U], mm_dt, tag="wv")  # [d_lo, dc, u]
            if mm_dt == f32:
                nc.sync.dma_start(
                    wv_sb[:], wv_d.rearrange("(c l) u -> l c u", l=128)
                )
            else:
                for dc in range(DC):
                    stg = wpool.tile([128, U], f32, tag="wvstg")
                    nc.sync.dma_start(stg[:], wv_d[dc * 128:(dc + 1) * 128, :])
                    nc.vector.tensor_copy(wv_sb[:, dc, :], stg[:])

            # ---------------- main loop over local batch rows ----------------
            for b in [bb % BL for bb in range(BL * repeat)]:
                vals_b = []
                for tt in range(NT):
                    vt = valsp.tile([128, C4, D], f32, tag="vals")
                    nc.sync.dma_start(
                        vt[:],
                        v_d[b, tt * TT:(tt + 1) * TT, :].rearrange(
                            "(c p) d -> p c d", p=128
                        ),
                    )
                    vals_b.append(vt)

                psc = psp.tile([128, TC], f32, tag="ps")  # score, column-major
                first_score = True
                for tt in range(NT):
                    # --- transpose values tile -> [d, t] ---
                    vT_tiles = []
                    for dc in range(DC):
                        ptr_t = ptrp.tile([128, TT], f32, tag="ptr")
                        for j in range(C4):
                            nc.tensor.matmul(
                                ptr_t[:, j * 128:(j + 1) * 128],
                                vals_b[tt][:, j, dc * 128:(dc + 1) * 128],
                                identity[:],
                                is_transpose=True,
                                start=(j == 0),
                                stop=(j == C4 - 1),
                            )
                        vTt = vTp.tile([128, TT], mm_dt, tag="vT")
                        nc.vector.tensor_copy(vTt[:], ptr_t[:])
                        vT_tiles.append(vTt)
                    # --- v_projT + tanh ---
                    hid_tiles = []
                    for uc in range(UC):
                        pvt = pvp.tile([128, TT], f32, tag="pv")
                        for dc in range(DC):
                            nc.tensor.matmul(
                                pvt[:],
                                wv_sb[:, dc, uc * 128:(uc + 1) * 128],
                                vT_tiles[dc][:],
                                start=(dc == 0),
                                stop=(dc == DC - 1),
                            )
                        ht = hidp.tile([128, TT], f32, tag="hid")
                        nc.scalar.activation(
                            ht[:], pvt[:], AF.Tanh, bias=bias[:, uc, b:b + 1]
                        )
                        hid_tiles.append(ht)
                    # --- score (N=1 matmuls, one PSUM group per row) ---
                    for j in range(C4):
                        for uc in range(UC):
                            nc.tensor.matmul(
                                psc[:, tt * C4 + j: tt * C4 + j + 1],
                                hid_tiles[uc][:, j * 128:(j + 1) * 128],
                                vw_sb[:, uc:uc + 1],
                                start=first_score,
                                stop=(
                                    tt == NT - 1 and j == C4 - 1 and uc == UC - 1
                                ),
                            )
                            first_score = False

                # --- masked softmax, column-major, no max subtraction ---
                masked = smallp.tile([128, TC], f32, tag="masked")
                nc.vector.tensor_add(masked[:], psc[:], addend[:, b, :])
                expc = smallp.tile([128, TC], f32, tag="expc")
                partial = smallp.tile([128, 1], f32, tag="partial")
                nc.scalar.activation(
                    expc[:], masked[:], AF.Exp, accum_out=partial[:]
                )
                ptot = pmp.tile([1, 1], f32, tag="pm")
                nc.tensor.matmul(
                    ptot[:], partial[:], ones_col[:], start=True, stop=True
                )
                recip = smallp.tile([1, 1], f32, tag="recip")
                nc.vector.reciprocal(recip[:], ptot[:])
                pbc = pmp.tile([128, 1], f32, tag="pm")
                nc.tensor.matmul(
                    pbc[:], ones_row[:], recip[:], start=True, stop=True
                )
                rbc = smallp.tile([128, 1], f32, tag="rbc")
                nc.vector.tensor_copy(rbc[:], pbc[:])
                wT = smallp.tile([128, TC], f32, tag="wT")
                nc.vector.tensor_scalar_mul(wT[:], expc[:], rbc[:])
                nc.sync.dma_start(
                    aw_d[b].rearrange("(c l) -> l c", l=128), wT[:]
                )

                # --- context ---
                pctx = pcp.tile([128, DC], f32, tag="pc")
                first = True
                for dc in range(DC):
                    for c in range(TC):
                        nc.tensor.matmul(
                            pctx[:, dc:dc + 1],
                            vals_b[c // C4][:, c % C4, dc * 128:(dc + 1) * 128],
                            wT[:, c:c + 1],
                            start=first,
                            stop=(dc == DC - 1 and c == TC - 1),
                        )
                        first = False
                ctx_sb = smallp.tile([128, DC], f32, tag="ctxsb")
                nc.vector.tensor_copy(ctx_sb[:], pctx[:])
                nc.sync.dma_start(
                    ctx_d[b].rearrange("(c l) -> l c", l=128), ctx_sb[:]
                )

    nc.compile()
    return nc


MM_DT_DEFAULT = "float32"


def get_nc():
    if "nc" not in _CACHE:
        import os
        _CACHE["nc"] = _build_nc(os.environ.get("MM_DT", MM_DT_DEFAULT))
    return _CACHE["nc"]


def make_in_maps(query, values, mask, Wq, bq, Wv, bv, Vw):
    in_maps = []
    for c in range(NCORES):
        sl = slice(c * BL, (c + 1) * BL)
        in_maps.append({
            "query": np.ascontiguousarray(query[sl]),
            "values": np.ascontiguousarray(values[sl]),
            "mask": np.ascontiguousarray(mask[sl]),
            "Wq": Wq, "bq": bq, "Wv": Wv, "bv": bv, "Vw": Vw,
        })
    return in_maps


def kernel(query, values, mask, Wq, bq, Wv, bv, Vw, Vb):
    from concourse.bass_utils import run_bass_kernel_spmd

    query = np.asarray(query, np.float32)
    values = np.asarray(values, np.float32)
    mask = np.asarray(mask, np.int32)
    Wq = np.asarray(Wq, np.float32)
    bq = np.asarray(bq, np.float32)
    Wv = np.asarray(Wv, np.float32)
    bv = np.asarray(bv, np.float32)
    Vw = np.asarray(Vw, np.float32)
    # Vb shifts every score equally -> softmax-invariant; not needed on device.

    nc = get_nc()
    in_maps = make_in_maps(query, values, mask, Wq, bq, Wv, bv, Vw)
    res = run_bass_kernel_spmd(nc, in_maps, list(range(NCORES))).results
    context = np.concatenate([r["context"] for r in res], axis=0)
    aw = np.concatenate([r["attention_weights"] for r in res], axis=0)
    return context, aw


# revision 35
# speedup vs baseline: 19.7815x; 1.7420x over previous
"""Additive attention (Bahdanau) Trainium2 Bass/Tile kernel.

Model (per batch row b):
    q_proj = query @ Wq + bq                       [U]
    v_proj = values[b] @ Wv + bv                   [T, U]
    hidden = tanh(q_proj + v_proj)                 [T, U]
    score  = hidden @ Vw (+ Vb; softmax-invariant) [T]
    score += (1 - mask) * -1e9
    w      = softmax(score)                        [T]
    ctx    = w @ values[b]                         [D]
returns (context [B, D], attention_weights [B, T])

Sharding: data-parallel over batch. B=32 rows -> 4 rows on each of the 8
NeuronCores; weights replicated. Every core runs the same NEFF (SPMD).

Per-core dataflow (BL=4 local rows):
  - values[b] loaded once (fp32, HWDGE) into natural [T-part, D-free] tiles,
    kept resident per row for the context matmul.
  - PE transposes 128x128 blocks -> PSUM; the DVE copy to SBUF rounds to
    float32r (f32r = TF32-like single-pass tensor-engine mode, ~1e-4 relative
    rounding, 4x the fp32 matmul rate).  Transpose-mode does not count as
    PE-busy for the HAM clock gate, so transpose units are interleaved
    between v_proj matmul groups via a software pump to keep the PE warm.
  - v_projT[u, t] = sum_dc Wv_chunk.T @ valuesT_chunk  (f32r, fp32 PSUM acc)
  - hiddenT = tanh(v_projT + bias_col) fused on ScalarE, emitted as f32r.
  - score rows [1, 512] = Vw_chunk.T @ hiddenT accumulate on PE (stationary
    operand is a single Vw column -> negligible weight-load cost), then tiny
    transpose-mode ops flip each row into the column-major score tile
    [128, 16] that the softmax wants.
  - softmax column-major WITHOUT max subtraction (|score| <= ~40 since
    |hidden|<=1 and Vw is small; masked lanes get -1e9 -> exp -> 0, matching
    the reference): Exp on ACT with accum_out per-partition sums, ones-vector
    matmul reduces across partitions, DVE reciprocal, second ones-matmul
    broadcasts it back; attention weights leave column-major via strided DMA.
  - context: N=1 matmuls ctxT[d,1] += values_chunk.T @ w_col over the
    resident fp32 tiles.
"""

import numpy as np

B, T, D, U = 32, 2048, 1024, 1024
NCORES = 8
BL = B // NCORES  # 4 local batch rows per core
NT = 4            # t-tiles per row, 512 wide each
TT = 512
C4 = TT // 128    # 128-chunks per t-tile
TC = T // 128     # 128-chunks per row (16)
DC = D // 128     # 8
UC = U // 128     # 8
NEG = -1.0e9

_CACHE = {}


def _build_nc(mm_dt_name="float32r", repeat=1, ablate=None):
    from collections import deque

    import concourse.bacc as bacc
    import concourse.tile as tile
    from concourse import mybir, masks

    f32 = mybir.dt.float32
    i32 = mybir.dt.int32
    mm_dt = getattr(mybir.dt, mm_dt_name)
    AF = mybir.ActivationFunctionType
    ALU = mybir.AluOpType

    nc = bacc.Bacc("TRN2", target_bir_lowering=False, debug=False)

    q_d = nc.dram_tensor("query", [BL, D], f32, kind="ExternalInput").ap()
    v_d = nc.dram_tensor("values", [BL, T, D], f32, kind="ExternalInput").ap()
    m_d = nc.dram_tensor("mask", [BL, T], i32, kind="ExternalInput").ap()
    wq_d = nc.dram_tensor("Wq", [D, U], f32, kind="ExternalInput").ap()
    bq_d = nc.dram_tensor("bq", [U], f32, kind="ExternalInput").ap()
    wv_d = nc.dram_tensor("Wv", [D, U], f32, kind="ExternalInput").ap()
    bv_d = nc.dram_tensor("bv", [U], f32, kind="ExternalInput").ap()
    vw_d = nc.dram_tensor("Vw", [U, 1], f32, kind="ExternalInput").ap()
    ctx_d = nc.dram_tensor("context", [BL, D], f32, kind="ExternalOutput").ap()
    aw_d = nc.dram_tensor(
        "attention_weights", [BL, T], f32, kind="ExternalOutput"
    ).ap()

    with tile.TileContext(nc) as tc:
        with (
            tc.tile_pool(name="const", bufs=1) as constp,
            tc.tile_pool(name="wpool", bufs=1) as wpool,
            tc.tile_pool(name="vals", bufs=5) as valsp,
            tc.tile_pool(name="vT", bufs=16) as vTp,
            tc.tile_pool(name="hid", bufs=9) as hidp,
            tc.tile_pool(name="small", bufs=2) as smallp,
            tc.tile_pool(name="ptr", bufs=2, space="PSUM") as ptrp,
            tc.tile_pool(name="pv", bufs=2, space="PSUM") as pvp,
            tc.tile_pool(name="ps", bufs=1, space="PSUM") as psp,
            tc.tile_pool(name="prow", bufs=1, space="PSUM") as prowp,
            tc.tile_pool(name="pc", bufs=1, space="PSUM") as pcp,
            tc.tile_pool(name="pm", bufs=1, space="PSUM") as pmp,
        ):
            # ---------------- constants / preamble ----------------
            identity = constp.tile([128, 128], f32)
            masks.make_identity(nc, identity[:])
            ones_col = constp.tile([128, 1], f32)
            nc.gpsimd.memset(ones_col[:], 1.0)
            ones_row = constp.tile([1, 128], f32)
            nc.gpsimd.memset(ones_row[:], 1.0)

            vw_sb = constp.tile([128, UC], f32)  # [u_lo, uc]
            nc.sync.dma_start(
                vw_sb[:], vw_d.rearrange("(c l) one -> l (c one)", l=128)
            )
            vw_r = constp.tile([128, UC], mm_dt)
            nc.vector.tensor_copy(vw_r[:], vw_sb[:])
            bqv = constp.tile([128, UC], f32)  # bq + bv, [u_lo, uc]
            tmpb = constp.tile([128, UC], f32)
            nc.sync.dma_start(bqv[:], bq_d.rearrange("(c l) -> l c", l=128))
            nc.sync.dma_start(tmpb[:], bv_d.rearrange("(c l) -> l c", l=128))
            nc.vector.tensor_add(bqv[:], bqv[:], tmpb[:])

            qT = constp.tile([128, DC, BL], f32)  # [d_lo, dc, b]
            for dc in range(DC):
                nc.sync.dma_start(
                    qT[:, dc, :],
                    q_d[:, dc * 128:(dc + 1) * 128].rearrange("b l -> l b"),
                )

            # mask addend, column-major per row: (mask - 1) * 1e9
            mcol_i = constp.tile([128, BL, TC], i32)
            for b in range(BL):
                nc.sync.dma_start(
                    mcol_i[:, b, :], m_d[b].rearrange("(c l) -> l c", l=128)
                )
            addend = constp.tile([128, BL, TC], f32)
            nc.vector.tensor_copy(addend[:], mcol_i[:])  # int32 -> fp32
            nc.vector.tensor_scalar(
                out=addend[:], in0=addend[:], scalar1=1.0e9, scalar2=NEG,
                op0=ALU.mult, op1=ALU.add,
            )

            # bias[u_lo, uc, b] = q_proj[b, u] + bq[u] + bv[u], via PE
            bias = constp.tile([128, UC, BL], f32)
            for uh in range(2):  # stream Wq in two halves to save SBUF
                wq_sb = wpool.tile([128, DC, U // 2], f32, tag="wq")
                nc.sync.dma_start(
                    wq_sb[:],
                    wq_d[:, uh * 512:(uh + 1) * 512].rearrange(
                        "(c l) u -> l c u", l=128
                    ),
                )
                for uq in range(UC // 2):
                    uc = uh * (UC // 2) + uq
                    pq = ptrp.tile([128, BL], f32, tag="ptr")
                    for dc in range(DC):
                        nc.tensor.matmul(
                            pq[:],
                            wq_sb[:, dc, uq * 128:(uq + 1) * 128],
                            qT[:, dc, :],
                            start=(dc == 0),
                            stop=(dc == DC - 1),
                        )
                    nc.vector.tensor_scalar_add(
                        bias[:, uc, :], pq[:], bqv[:, uc:uc + 1]
                    )

            # Wv as matmul stationary operand in mm_dt ([d_lo, dc, u]).
            # f32r operands must come from a producer that *rounds* to f32r
            # (BIR verifier rule) -> stage fp32 chunks through a DVE cast.
            wv_sb = wpool.tile([128, DC, U], mm_dt, tag="wv")
            if mm_dt == f32:
                nc.sync.dma_start(
                    wv_sb[:], wv_d.rearrange("(c l) u -> l c u", l=128)
                )
            else:
                for dc in range(DC):
                    stg = wpool.tile([128, U], f32, tag="wvstg")
                    nc.sync.dma_start(stg[:], wv_d[dc * 128:(dc + 1) * 128, :])
                    nc.vector.tensor_copy(wv_sb[:, dc, :], stg[:])

            # ------------- transpose-unit pump (software pipeline) ---------
            # Each unit: 4 transpose-mode matmuls into one PSUM bank + one
            # DVE copy (which rounds to f32r) to an SBUF valuesT tile.
            # Units are emitted interleaved between v_proj matmul groups so
            # the PE never sits in long transpose-only stretches
            # (transpose-mode doesn't count as PE-busy for the HAM gate).
            trans_q = deque()   # (key, vals_tile, dc, sink dict)
            vT_map = {}         # key=(rep,b,tt) -> dict dc -> vT tile

            def emit_unit():
                key, vtile, dc, sink = trans_q.popleft()
                if ablate == "notr":
                    vTt = vTp.tile([128, TT], mm_dt, tag="vT")
                    nc.vector.tensor_copy(vTt[:], vtile[:, 0, 0:TT])
                    sink[dc] = vTt
                    return
                ptr_t = ptrp.tile([128, TT], f32, tag="ptr")
                for j in range(C4):
                    nc.tensor.matmul(
                        ptr_t[:, j * 128:(j + 1) * 128],
                        vtile[:, j, dc * 128:(dc + 1) * 128],
                        identity[:],
                        is_transpose=True,
                        start=(j == 0),
                        stop=(j == C4 - 1),
                    )
                vTt = vTp.tile([128, TT], mm_dt, tag="vT")
                nc.vector.tensor_copy(vTt[:], ptr_t[:])
                sink[dc] = vTt

            def pump(n=1):
                # Interleaving transpose units between matmul groups measured
                # *slower* on HW (PE blocks in-order on PSUM-slot waits that
                # depend on DVE copies), so the pump is off unless requested.
                if ablate != "interleave":
                    return
                for _ in range(n):
                    if trans_q:
                        emit_unit()

            def ensure_done(key):
                while trans_q and trans_q[0][0] <= key:
                    emit_unit()

            def push_batch(rep, b, vals_b):
                for tt in range(NT):
                    sink = {}
                    vT_map[(rep, b, tt)] = sink
                    for dc in range(DC):
                        trans_q.append(((rep, b, tt), vals_b[tt], dc, sink))

            def load_batch(b):
                tiles = []
                for tt in range(NT):
                    vt = valsp.tile([128, C4, D], f32, tag="vals")
                    nc.sync.dma_start(
                        vt[:],
                        v_d[b, tt * TT:(tt + 1) * TT, :].rearrange(
                            "(c p) d -> p c d", p=128
                        ),
                    )
                    tiles.append(vt)
                return tiles

            # ---------------- main loop over local batch rows --------------
            batch_seq = [
                (rep, bb) for rep in range(repeat) for bb in range(BL)
            ]
            vals_map = {}
            vals_map[batch_seq[0]] = load_batch(batch_seq[0][1])
            push_batch(*batch_seq[0], vals_map[batch_seq[0]])

            for bi, (rep, b) in enumerate(batch_seq):
                vals_b = vals_map.pop((rep, b))
                psc = psp.tile([128, TC], f32, tag="ps")  # score columns
                for tt in range(NT):
                    ensure_done((rep, b, tt))
                    # --- v_projT + tanh, transposes pumped in between ---
                    vT_tiles = vT_map.pop((rep, b, tt))
                    hid_tiles = []
                    n_dc = 1 if ablate == "vproj1" else DC
                    for uc in range(UC):
                        pvt = pvp.tile([128, TT], f32, tag="pv")
                        for dc in range(n_dc):
                            nc.tensor.matmul(
                                pvt[:],
                                wv_sb[:, dc, uc * 128:(uc + 1) * 128],
                                vT_tiles[dc][:],
                                start=(dc == 0),
                                stop=(dc == n_dc - 1),
                            )
                        ht = hidp.tile([128, TT], mm_dt, tag="hid")
                        nc.scalar.activation(
                            ht[:], pvt[:], AF.Tanh, bias=bias[:, uc, b:b + 1]
                        )
                        hid_tiles.append(ht)
                        pump(1)
                    # --- score row [1, 512]: stationary = Vw column ---
                    n_uc = 1 if ablate == "mini" else UC
                    prow = prowp.tile([1, TT], f32, tag="prow")
                    for uc in range(n_uc):
                        nc.tensor.matmul(
                            prow[:],
                            vw_r[:, uc:uc + 1],
                            hid_tiles[uc][:],
                            start=(uc == 0),
                            stop=(uc == n_uc - 1),
                        )
                    srow = smallp.tile([1, TT], f32, tag="srow")
                    nc.vector.tensor_copy(srow[:], prow[:])
                    # flip the row into score columns via tiny transposes
                    for j in range(C4):
                        nc.tensor.matmul(
                            psc[:, tt * C4 + j: tt * C4 + j + 1],
                            srow[:, j * 128:(j + 1) * 128],
                            ones_row[:1, :1],
                            is_transpose=True,
                            start=True,
                            stop=True,
                        )

                # prefetch next batch before this batch's tail
                if bi + 1 < len(batch_seq):
                    nkey = batch_seq[bi + 1]
                    vals_map[nkey] = load_batch(nkey[1])
                    push_batch(*nkey, vals_map[nkey])

                # --- masked softmax, column-major, no max subtraction ---
                masked = smallp.tile([128, TC], f32, tag="masked")
                nc.vector.tensor_add(masked[:], psc[:], addend[:, b, :])
                expc = smallp.tile([128, TC], f32, tag="expc")
                partial = smallp.tile([128, 1], f32, tag="partial")
                nc.scalar.activation(
                    expc[:], masked[:], AF.Exp, accum_out=partial[:]
                )
                ptot = pmp.tile([1, 1], f32, tag="pm")
                nc.tensor.matmul(
                    ptot[:], partial[:], ones_col[:], start=True, stop=True
                )
                recip = smallp.tile([1, 1], f32, tag="recip")
                nc.vector.reciprocal(recip[:], ptot[:])
                pbc = pmp.tile([128, 1], f32, tag="pm")
                nc.tensor.matmul(
                    pbc[:], ones_row[:], recip[:], start=True, stop=True
                )
                rbc = smallp.tile([128, 1], f32, tag="rbc")
                nc.vector.tensor_copy(rbc[:], pbc[:])
                wT = smallp.tile([128, TC], f32, tag="wT")
                nc.vector.tensor_scalar_mul(wT[:], expc[:], rbc[:])
                nc.sync.dma_start(
                    aw_d[b].rearrange("(c l) -> l c", l=128), wT[:]
                )

                # --- context: ctxT[d,1] += values_chunk.T @ w_col ---
                pctx = pcp.tile([128, DC], f32, tag="pc")
                first = True
                n_tc = 2 if ablate == "mini" else TC
                for dc in range(DC):
                    for c in range(n_tc):
                        nc.tensor.matmul(
                            pctx[:, dc:dc + 1],
                            vals_b[c // C4][:, c % C4, dc * 128:(dc + 1) * 128],
                            wT[:, c:c + 1],
                            start=first,
                            stop=(dc == DC - 1 and c == n_tc - 1),
                        )
                        first = False
                    pump(1)
                ctx_sb = smallp.tile([128, DC], f32, tag="ctxsb")
                nc.vector.tensor_copy(ctx_sb[:], pctx[:])
                nc.sync.dma_start(
                    ctx_d[b].rearrange("(c l) -> l c", l=128), ctx_sb[:]
                )

    nc.compile()
    return nc


MM_DT_DEFAULT = "float32r"


def get_nc():
    if "nc" not in _CACHE:
        import os
        _CACHE["nc"] = _build_nc(os.environ.get("MM_DT", MM_DT_DEFAULT))
    return _CACHE["nc"]


def make_in_maps(query, values, mask, Wq, bq, Wv, bv, Vw):
    in_maps = []
    for c in range(NCORES):
        sl = slice(c * BL, (c + 1) * BL)
        in_maps.append({
            "query": np.ascontiguousarray(query[sl]),
            "values": np.ascontiguousarray(values[sl]),
            "mask": np.ascontiguousarray(mask[sl]),
            "Wq": Wq, "bq": bq, "Wv": Wv, "bv": bv, "Vw": Vw,
        })
    return in_maps


def kernel(query, values, mask, Wq, bq, Wv, bv, Vw, Vb):
    from concourse.bass_utils import run_bass_kernel_spmd

    query = np.asarray(query, np.float32)
    values = np.asarray(values, np.float32)
    mask = np.asarray(mask, np.int32)
    Wq = np.asarray(Wq, np.float32)
    bq = np.asarray(bq, np.float32)
    Wv = np.asarray(Wv, np.float32)
    bv = np.asarray(bv, np.float32)
    Vw = np.asarray(Vw, np.float32)
    # Vb shifts every score equally -> softmax-invariant; not needed on device.

    nc = get_nc()
    in_maps = make_in_maps(query, values, mask, Wq, bq, Wv, bv, Vw)
    res = run_bass_kernel_spmd(nc, in_maps, list(range(NCORES))).results
    context = np.concatenate([r["context"] for r in res], axis=0)
    aw = np.concatenate([r["attention_weights"] for r in res], axis=0)
    return context, aw
